# revision 15
# baseline (speedup 1.0000x reference)
"""MetaPathGNN forward on 8 Trainium2 NeuronCores (Bass/Tile).

Layout strategy (SPMD, one program on 8 cores):
  - Nodes sharded by id: core c owns rows [c*12500, (c+1)*12500).
  - Host: x is pre-transposed per shard (xT [128, 12500]); edges are routed to
    their destination core, then packed into 128-edge tiles that are pure in
    (destination 128-node subchunk, source 25k-row bank). Bank purity lets the
    h[col] gather run through the int16-indexed dma_gather Q7 instruction
    (4 banks x 25000 rows); subchunk purity makes the segment-sum a one-hot
    matmul accumulating agg.T in PSUM at compile-time column offsets.
  - Device: 3-layer MLP in fp32 (h.T layout), w0/w1 output terms fused in the
    same pass; h cast to bf16, AllGather to replicate h; banked bf16 gathers;
    per-tile one-hot S (DVE is_equal vs iota) and PE matmuls reduce edges;
    wl matmul + bias + ReLU, PE transpose back to row-major, store.
"""
import numpy as np
import ml_dtypes

from concourse import bass, bacc, mybir, tile, bass_utils
from concourse.masks import make_identity

NCORES = 8
N_NODES = 100000
N_EDGES = 600000
D = 128          # input/hidden*2 dim
HID = 64
NPC = N_NODES // NCORES          # 12500 nodes per core
CHUNK = 500
NCH = NPC // CHUNK               # 25 chunks
NSC = 4                          # subchunks per chunk
SLOTW = 125                      # nodes per subchunk (slot width)
NBANK = 4
BANKROWS = N_NODES // NBANK      # (legacy name; banks now split-AG based)
SPLITA = 6500                    # rows per core in first AllGather half (13 chunks)
SPLITB = NPC - SPLITA            # 6500 rows (13 chunks)
SPLCH = SPLITA // CHUNK          # 12 chunks in half A
BANKA = NCORES * SPLITA // 2     # 24000 rows per bank in table A
BANKB = NCORES * SPLITB // 2     # 26000 rows per bank in table B
TSB = 2                          # tiles per (subchunk, bank)
TPC = NSC * TSB                  # tiles per (chunk, bank) = 8
NIDX = TPC * 128                 # 1024 indices per dma_gather call
NCALL = NCH * NBANK              # 100 gather calls per core
GCOLS = NCALL * TPC              # 800 tile columns
IDXW = NCALL * (NIDX // 16)      # 6400 int16 idx columns
PAD_LS = 300.0                   # one-hot miss marker for padded edges

F32 = mybir.dt.float32
BF16 = mybir.dt.bfloat16
I16 = mybir.dt.int16
F32R = mybir.dt.float32r

_CACHE = {}


def _mm32r(nc, out, lhsT, rhs, start=True, stop=True):
    """fp32 matmul in float32r mode (1 cyc/col at N>=256 vs 4 for fp32)."""
    import concourse.mybir as _mb
    nc.tensor.matmul(out, lhsT.bitcast(_mb.dt.float32r), rhs.bitcast(_mb.dt.float32r),
                     start=start, stop=stop)



def _build():
    nc = bacc.Bacc("TRN2", target_bir_lowering=False, debug=False,
                   num_devices=NCORES, num_swdge_queues=4)
    xT = nc.dram_tensor("xT", [D, NPC], F32, kind="ExternalInput")
    gidx = nc.dram_tensor("gidx", [128, IDXW], I16, kind="ExternalInput")
    gls = nc.dram_tensor("gls", [128, GCOLS], BF16, kind="ExternalInput")
    iot = nc.dram_tensor("iot", [128, TPC * 128], BF16, kind="ExternalInput")
    w1 = nc.dram_tensor("w1", [D, HID], F32, kind="ExternalInput")
    b1 = nc.dram_tensor("b1", [HID, 1], F32, kind="ExternalInput")
    w2 = nc.dram_tensor("w2", [HID, HID], F32, kind="ExternalInput")
    b2 = nc.dram_tensor("b2", [HID, 1], F32, kind="ExternalInput")
    w3 = nc.dram_tensor("w3", [HID, D], F32, kind="ExternalInput")
    b3 = nc.dram_tensor("b3", [D, 1], F32, kind="ExternalInput")
    wl = nc.dram_tensor("wl", [D, HID], F32, kind="ExternalInput")
    w0 = nc.dram_tensor("w0", [D, HID], F32, kind="ExternalInput")
    w1b = nc.dram_tensor("w1b", [D, HID], F32, kind="ExternalInput")
    fb = nc.dram_tensor("fb", [HID, 1], F32, kind="ExternalInput")
    out = nc.dram_tensor("out", [HID, NPC], F32, kind="ExternalOutput")

    with tile.TileContext(nc) as tc:
        with (
            tc.tile_pool(name="dram", bufs=1, space="DRAM") as dram,
            tc.tile_pool(name="const", bufs=1) as cp,
            tc.tile_pool(name="sb", bufs=3) as sb,
            tc.tile_pool(name="gtp", bufs=5) as gtp,
            tc.tile_pool(name="sbS", bufs=2) as sbS,
            tc.tile_pool(name="ps", bufs=1, space="PSUM") as ps,
            tc.tile_pool(name="ps2", bufs=2, space="PSUM") as ps2,
            tc.tile_pool(name="ps3", bufs=2, space="PSUM") as ps3,
        ):
            h_loc_a = dram.tile([SPLITA, D], BF16)
            h_loc_b = dram.tile([SPLITB, D], BF16)
            h_half_a = dram.tile([NCORES * SPLITA, D], BF16, addr_space="Shared")
            h_half_b = dram.tile([NCORES * SPLITB, D], BF16, addr_space="Shared")

            # constants / weights
            w1s = cp.tile([D, HID], F32); nc.sync.dma_start(w1s[:], w1[:, :])
            w2s = cp.tile([HID, HID], F32); nc.sync.dma_start(w2s[:], w2[:, :])
            w3s = cp.tile([HID, D], F32); nc.sync.dma_start(w3s[:], w3[:, :])
            wls = cp.tile([D, HID], F32); nc.sync.dma_start(wls[:], wl[:, :])
            w0s = cp.tile([D, HID], F32); nc.sync.dma_start(w0s[:], w0[:, :])
            w1bs = cp.tile([D, HID], F32); nc.sync.dma_start(w1bs[:], w1b[:, :])
            b1s = cp.tile([HID, 1], F32); nc.sync.dma_start(b1s[:], b1[:, :])
            b2s = cp.tile([HID, 1], F32); nc.sync.dma_start(b2s[:], b2[:, :])
            b3s = cp.tile([D, 1], F32); nc.sync.dma_start(b3s[:], b3[:, :])
            fbs = cp.tile([HID, 1], F32); nc.sync.dma_start(fbs[:], fb[:, :])
            iots = cp.tile([128, TPC * 128], BF16); nc.sync.dma_start(iots[:], iot[:, :])
            idx_t = cp.tile([128, IDXW], I16); nc.sync.dma_start(idx_t[:], gidx[:, :])
            ls_t = cp.tile([128, GCOLS], BF16); nc.sync.dma_start(ls_t[:], gls[:, :])
            ident = cp.tile([128, 128], F32); make_identity(nc, ident[:])
            w2r = cp.tile([HID, HID], F32R); nc.vector.tensor_copy(w2r[:], w2s[:])
            w1r = cp.tile([D, HID], F32R); nc.vector.tensor_copy(w1r[:], w1s[:])
            w1br = cp.tile([D, HID], F32R); nc.vector.tensor_copy(w1br[:], w1bs[:])
            w0r = cp.tile([D, HID], F32R); nc.vector.tensor_copy(w0r[:], w0s[:])
            w3r = cp.tile([HID, D], F32R); nc.vector.tensor_copy(w3r[:], w3s[:])
            wlr = cp.tile([D, HID], F32R); nc.vector.tensor_copy(wlr[:], wls[:])
            partial = cp.tile([HID, NPC], F32)

            # ---- Phase A: MLP + w0/w1 partial + h store (bf16) ----
            for ch in range(NCH):
                cs = ch * CHUNK
                xt = sb.tile([D, CHUNK], F32, tag="xt")
                nc.sync.dma_start(xt[:], xT[:, cs:cs + CHUNK])
                xtr = sb.tile([D, CHUNK], F32R, tag="xtr")
                nc.vector.tensor_copy(xtr[:], xt[:])
                p1 = ps.tile([HID, CHUNK], F32, tag="p1")
                nc.tensor.matmul(p1[:], w1r[:], xtr[:], start=True, stop=True)
                h1 = sb.tile([HID, CHUNK], F32R, tag="h1")
                nc.scalar.activation(h1[:], p1[:], mybir.ActivationFunctionType.Relu, bias=b1s[:])
                p2 = ps.tile([HID, CHUNK], F32, tag="p2")
                nc.tensor.matmul(p2[:], w2r[:], h1[:], start=True, stop=True)
                h2 = sb.tile([HID, CHUNK], F32R, tag="h2")
                nc.scalar.activation(h2[:], p2[:], mybir.ActivationFunctionType.Relu, bias=b2s[:])
                p3 = ps.tile([D, CHUNK], F32, tag="p3")
                nc.tensor.matmul(p3[:], w3r[:], h2[:], start=True, stop=True)
                h3 = sb.tile([D, CHUNK], F32, tag="h3")
                nc.scalar.activation(h3[:], p3[:], mybir.ActivationFunctionType.Identity, bias=b3s[:])
                h3r = sb.tile([D, CHUNK], F32R, tag="h3r")
                nc.vector.tensor_copy(h3r[:], h3[:])
                pp = ps.tile([HID, CHUNK], F32, tag="pp")
                nc.tensor.matmul(pp[:], w0r[:], h3r[:], start=True, stop=False)
                nc.tensor.matmul(pp[:], w1br[:], xtr[:], start=False, stop=True)
                nc.vector.tensor_copy(partial[:, cs:cs + CHUNK], pp[:])
                hb = sb.tile([128, NSC * D], BF16, tag="hb")
                for j in range(NSC):
                    tp = ps2.tile([128, 128], F32, tag="tp")
                    nc.tensor.transpose(tp[:SLOTW, :], h3[:, j * SLOTW:(j + 1) * SLOTW], ident[:])
                    if j % 2 == 0:
                        nc.vector.tensor_copy(hb[:SLOTW, j * D:(j + 1) * D], tp[:SLOTW, :])
                    else:
                        nc.scalar.activation(hb[:SLOTW, j * D:(j + 1) * D], tp[:SLOTW, :],
                                             mybir.ActivationFunctionType.Copy)
                if ch < SPLCH:
                    dst = h_loc_a[cs:cs + CHUNK, :]
                else:
                    dst = h_loc_b[cs - SPLITA:cs - SPLITA + CHUNK, :]
                nc.sync.dma_start(
                    dst.rearrange("(p j) d -> p (j d)", p=SLOTW),
                    hb[:SLOTW, :])
                if ch == SPLCH - 1:
                    # first AllGather overlaps the remaining MLP chunks
                    nc.gpsimd.collective_compute(
                        "AllGather", mybir.AluOpType.bypass,
                        replica_groups=[list(range(NCORES))],
                        ins=[h_loc_a.opt()], outs=[h_half_a.opt()],
                    )

            nc.gpsimd.collective_compute(
                "AllGather", mybir.AluOpType.bypass,
                replica_groups=[list(range(NCORES))],
                ins=[h_loc_b.opt()], outs=[h_half_b.opt()],
            )

            # ---- Phase C/D: gather, segment matmul, output ----
            for ch in range(NCH):
                cs = ch * CHUNK
                gts = []
                for b in range(NBANK):
                    call = ch * NBANK + b
                    gt = gtp.tile([128, TPC * D], BF16, tag=f"gt{b}")
                    nc.gpsimd.dma_gather(
                        out_ap=gt[:].rearrange("p (g d) -> p g d", d=D),
                        in_ap=(h_half_a[b * BANKA:(b + 1) * BANKA, :] if b < 2
                               else h_half_b[(b - 2) * BANKB:(b - 1) * BANKB, :]),
                        idxs_ap=idx_t[:, call * (NIDX // 16):(call + 1) * (NIDX // 16)],
                        num_idxs=NIDX, num_idxs_reg=NIDX, elem_size=D,
                        queue_num=b,
                    )
                    gts.append(gt)
                # pre-build all 32 one-hot S blocks for this chunk (hoistable
                # ahead of the gathers; bufs=2 overlaps with previous chunk)
                S_all = sbS.tile([128, NBANK * TPC * 128], BF16, tag="S")
                for sc in range(NSC):
                    base = ch * (NSC * NBANK * TSB) + sc * (NBANK * TSB)
                    nc.vector.tensor_tensor(
                        out=S_all[:, sc * (NBANK * TSB) * 128:(sc + 1) * (NBANK * TSB) * 128]
                            .rearrange("p (t d) -> p t d", d=128),
                        in0=ls_t[:, base:base + NBANK * TSB].to_broadcast([128, NBANK * TSB, 128]),
                        in1=iots[:].rearrange("p (t d) -> p t d", d=128),
                        op=mybir.AluOpType.is_equal)
                pa = ps3.tile([128, CHUNK], F32, tag="pa")
                for sc in range(NSC):
                    nmm = 0
                    for b in range(NBANK):
                        for j in range(TSB):
                            tl = sc * TSB + j
                            si = sc * (NBANK * TSB) + b * TSB + j
                            nc.tensor.matmul(
                                pa[:, sc * SLOTW:(sc + 1) * SLOTW],
                                gts[b][:, tl * D:(tl + 1) * D],
                                S_all[:, si * 128:si * 128 + SLOTW],
                                start=(nmm == 0), stop=(nmm == NBANK * TSB - 1))
                            nmm += 1
                aggT = sb.tile([128, CHUNK], F32R, tag="aggT")
                nc.scalar.activation(aggT[:], pa[:], mybir.ActivationFunctionType.Copy)
                po = ps.tile([HID, CHUNK], F32, tag="p1")
                nc.tensor.matmul(po[:], wlr[:], aggT[:], start=True, stop=True)
                ot = sb.tile([HID, CHUNK], F32, tag="ot")
                nc.vector.tensor_tensor(out=ot[:], in0=po[:],
                                        in1=partial[:, cs:cs + CHUNK],
                                        op=mybir.AluOpType.add)
                otr = sb.tile([HID, CHUNK], F32, tag="otr")
                nc.scalar.activation(otr[:], ot[:], mybir.ActivationFunctionType.Relu, bias=fbs[:])
                nc.sync.dma_start(out[:, cs:cs + CHUNK], otr[:])
    nc.compile()
    return nc


def _prep(inputs):
    """Host-side edge routing + per-core input maps."""
    x = np.asarray(inputs["x"], np.float32)
    ei = np.asarray(inputs["edge_index"])
    row = ei[0, 0].astype(np.int64)
    col = ei[0, 1].astype(np.int64)

    core = row // NPC
    er = row - core * NPC
    ch = er // CHUNK
    sc = (er % CHUNK) // SLOTW
    slot = er % SLOTW
    scol = col // NPC                 # source core
    rcol = col - scol * NPC
    oo = rcol % CHUNK
    rperm = (rcol // CHUNK) * CHUNK + (oo % SLOTW) * NSC + oo // SLOTW
    in_a = rcol < SPLITA
    rowp = np.where(in_a, scol * SPLITA + rperm, scol * SPLITB + (rperm - SPLITA))
    bank = np.where(in_a, rowp // BANKA, 2 + rowp // BANKB)
    brow = np.where(in_a, rowp % BANKA, rowp % BANKB).astype(np.int64)

    # group id: (core, ch, sc, bank)
    g = ((core * NCH + ch) * NSC + sc) * NBANK + bank
    ngroups = NCORES * NCH * NSC * NBANK
    order = np.argsort(g, kind="stable")
    gs = g[order]
    brow_s = brow[order]
    slot_s = slot[order]
    counts = np.bincount(gs, minlength=ngroups)
    if counts.max() > TSB * 128:
        raise ValueError(f"group overflow: {counts.max()} > {TSB*128}")
    starts = np.zeros(ngroups, np.int64)
    starts[1:] = np.cumsum(counts)[:-1]
    rank = np.arange(N_EDGES) - starts[gs]

    # flat position per edge inside its core's gather stream:
    # core stream = [call(ch,b)][pos], call = ch*NBANK+b, pos = sc*TSB*128 + rank
    g_core = gs // (NCH * NSC * NBANK)
    g_ch = (gs // (NSC * NBANK)) % NCH
    g_sc = (gs // NBANK) % NSC
    g_b = gs % NBANK
    pos = (g_ch * NBANK + g_b) * NIDX + g_sc * (TSB * 128) + rank

    idx_all = np.zeros((NCORES, NCH * NBANK * NIDX), np.int16)
    ls_all = np.full((NCORES, GCOLS, 128), PAD_LS, np.float32)
    for c in range(NCORES):
        m = g_core == c
        idx_all[c, pos[m]] = brow_s[m].astype(np.int16)
        # tile column + partition of each edge
        p_edge = pos[m] % 128
        tcol = pos[m] // 128
        ls_all[c, tcol, p_edge] = slot_s[m].astype(np.float32)

    # wrap idx: per call of 1024, i -> [i%16, i//16], replicate x8 partitions
    idx_w = np.zeros((NCORES, 128, IDXW), np.int16)
    for c in range(NCORES):
        a = idx_all[c].reshape(NCALL, NIDX // 16, 16)   # [call, i//16, i%16]
        blk = a.transpose(2, 0, 1).reshape(16, IDXW)    # [i%16, call-major i//16]
        idx_w[c] = np.tile(blk, (8, 1))

    # permute ls columns: old (ch*4+b)*8 + sc*2+j  ->  new ch*32 + sc*8 + b*2 + j
    o = np.arange(GCOLS)
    och = o // (NBANK * TPC); r = o % (NBANK * TPC)
    ob = r // TPC; ot = r % TPC
    osc = ot // TSB; oj = ot % TSB
    newcol = och * (NBANK * TPC) + osc * (NBANK * TSB) + ob * TSB + oj
    ls_perm = np.empty_like(ls_all)
    ls_perm[:, newcol, :] = ls_all[:, o, :]
    ls_w = ls_perm.transpose(0, 2, 1).astype(ml_dtypes.bfloat16)  # [core, 128, GCOLS]

    xT_all = np.ascontiguousarray(
        x.reshape(NCORES, NPC, D).transpose(0, 2, 1))  # [core, 128, NPC]

    iot_host = np.tile(np.arange(128, dtype=np.float32), (128, TPC)).astype(ml_dtypes.bfloat16)

    w = {k: np.asarray(inputs[k], np.float32) for k in
         ["mlp_w1", "mlp_b1", "mlp_w2", "mlp_b2", "mlp_w3", "mlp_b3",
          "wl_w", "wl_b", "w0_w", "w0_b", "w1_w", "w1_b"]}
    fused_b = (w["wl_b"] + w["w0_b"] + w["w1_b"]).reshape(HID, 1)

    in_maps = []
    for c in range(NCORES):
        in_maps.append({
            "xT": xT_all[c],
            "gidx": idx_w[c],
            "gls": np.ascontiguousarray(ls_w[c]),
            "iot": iot_host,
            "w1": w["mlp_w1"], "b1": w["mlp_b1"].reshape(HID, 1),
            "w2": w["mlp_w2"], "b2": w["mlp_b2"].reshape(HID, 1),
            "w3": w["mlp_w3"], "b3": w["mlp_b3"].reshape(D, 1),
            "wl": w["wl_w"], "w0": w["w0_w"], "w1b": w["w1_w"],
            "fb": fused_b,
        })
    return in_maps


def kernel(**inputs) -> np.ndarray:
    if "nc" not in _CACHE:
        _CACHE["nc"] = _build()
    nc = _CACHE["nc"]
    in_maps = _prep(inputs)
    res = bass_utils.run_bass_kernel_spmd(nc, in_maps, core_ids=list(range(NCORES)))
    return np.concatenate(
        [np.ascontiguousarray(res.results[c]["out"].T) for c in range(NCORES)],
        axis=0)

